# revision 11
# baseline (speedup 1.0000x reference)
"""Trainium2 Bass kernel for BeliefTreeMemory GNN message passing.

Strategy (8 NeuronCores, SPMD, one program):
  - Shard by tgt-node range: core c owns local nodes [0, 25000) = global
    [c*25000, (c+1)*25000).  Edges live on the core owning their tgt.
  - Edges sorted by tgt and grouped by 512-node tiles; per-tile counts
    padded to the max over cores (uniform compile-time structure).
  - h shard kept resident in SBUF, transposed [D, NSH_pad]:
      * tgt-reads  = gpsimd.ap_gather per 128-edge chunk (direct [D,E]).
      * GRU reads/writes the slab in place, tile by tile.
  - Aggregation: indicator matmul  aggT[D, 512] += m'^T @ S  accumulated
    in PSUM across a tile's chunks; S built on DVE by iota==tlocrel.
    recip[tgt] folded into m' (ACT scale); b2 folded into the xT copy.
  - GRU fully in transposed layout; has_msg mask folded as a rank-1
    K=1 matmul adding BIG*(1-has_msg) to the z-gate pre-activation.
  - Pass-1 h_src comes pre-gathered AND pre-transposed from the host
    (h0 is known); pass-2 h_src gathered from the AllGather output via
    per-chunk indirect DMA (128 rows/call).
  - One AllGather of row-major h1 shards between the passes.
"""

import sys
import numpy as np

sys.path.insert(0, "/opt/trn_rl_repo")

N_NODES = 200000
N_EDGES = 400000
D = 128
N_CORES = 8
N_PASSES = 2
TN = 512            # node-tile (and GRU chunk) size
BIG = 30.0


class _Cfg:
    def __init__(self, n_nodes=N_NODES, n_edges=N_EDGES):
        assert n_nodes % N_CORES == 0
        self.N = n_nodes
        self.E = n_edges
        self.NSH = n_nodes // N_CORES
        self.NSHP = ((self.NSH + TN - 1) // TN) * TN
        self.NTILES = self.NSHP // TN


def _rup(x, m):
    return ((x + m - 1) // m) * m


def _plan(cfg, inputs):
    """Host-side: per-core token layout + all swizzled input images."""
    N, NSH = cfg.N, cfg.NSH
    C = N_CORES
    src = np.asarray(inputs["src"]).astype(np.int64)
    tgt = np.asarray(inputs["tgt"]).astype(np.int64)
    etype = np.asarray(inputs["etype"]).astype(np.int64)
    cred = np.asarray(inputs["cred"], np.float32)
    h0 = np.asarray(inputs["h"], np.float32)
    E = src.shape[0]

    cnt = np.bincount(tgt, minlength=N).astype(np.int64)
    recip = (1.0 / np.maximum(cnt, 1)).astype(np.float32)

    core_of = tgt // NSH
    tloc = tgt - core_of * NSH
    tile_of = tloc // TN

    # per (core, tile) counts -> uniform padded sizes
    counts = np.zeros((C, cfg.NTILES), np.int64)
    for c in range(C):
        m = core_of == c
        counts[c] = np.bincount(tile_of[m], minlength=cfg.NTILES)
    P = np.array([_rup(max(int(counts[:, t].max()), 1), 128)
                  for t in range(cfg.NTILES)], np.int64)
    E_pad = int(P.sum())
    tile_of_chunk = np.repeat(np.arange(cfg.NTILES), P // 128)
    n_chunks = E_pad // 128

    recip_e = recip[tgt]
    cred_e = cred[src]

    per_core = []
    for c in range(C):
        SRC = np.zeros(E_pad, np.int64)
        TLOCREL = np.full(E_pad, -1.0, np.float32)
        TLOC = np.zeros(E_pad, np.int64)       # apg idx (tile-relative)
        REC = np.zeros(E_pad, np.float32)
        SRH = np.zeros((5, E_pad), np.float32)
        m = core_of == c
        eids = np.nonzero(m)[0]
        order = np.argsort(tloc[eids], kind="stable")
        eids = eids[order]
        et = tile_of[eids]
        off = 0
        for t in range(cfg.NTILES):
            ids = eids[et == t]
            nv = len(ids)
            sl = slice(off, off + nv)
            SRC[sl] = src[ids]
            TLOCREL[sl] = (tloc[ids] - t * TN).astype(np.float32)
            TLOC[sl] = tloc[ids] - t * TN
            REC[sl] = recip_e[ids]
            SRH[0, sl] = cred_e[ids]
            oh = np.eye(4, dtype=np.float32)[etype[ids]]
            SRH[1:5, sl] = oh.T
            off += int(P[t])
        assert off == E_pad

        # images
        srcg_img = SRC.reshape(n_chunks, 128).T.astype(np.int32)  # [128, nch]
        tlr_img = TLOCREL.reshape(n_chunks, 128).T.copy()         # [128, nch]
        rec_img = REC.reshape(n_chunks, 128).T.copy()             # [128, nch]
        # apg idx image: per chunk [128, 8]; row p, col s = u[s*16 + p%16]
        apg = np.zeros((128, 8 * n_chunks), np.int16)
        for ch in range(n_chunks):
            u = TLOC[ch * 128:(ch + 1) * 128]
            blk = u.reshape(8, 16).T.astype(np.int16)
            apg[:, 8 * ch:8 * (ch + 1)] = np.tile(blk, (8, 1))
        # pass-1 src feed, pre-transposed per chunk: [nch, 128 D, 128 E]
        feed = h0[SRC].reshape(n_chunks, 128, D).transpose(0, 2, 1)
        feed = np.ascontiguousarray(feed, np.float32)

        hshT = np.zeros((D, cfg.NSHP), np.float32)
        hshT[:, :NSH] = h0[c * NSH:(c + 1) * NSH].T
        nomsg = np.zeros((1, cfg.NSHP), np.float32)
        nomsg[0, :NSH] = BIG * (cnt[c * NSH:(c + 1) * NSH] == 0)
        nomsg[0, NSH:] = BIG

        per_core.append(dict(
            srcg=srcg_img, tlr=tlr_img, recg=rec_img, apg=apg,
            srhs=np.ascontiguousarray(SRH), feed=feed.reshape(-1),
            hsh0T=hshT, nomsg=nomsg))

    W1 = np.asarray(inputs["W1"], np.float32)
    ee = np.asarray(inputs["edge_emb"], np.float32)
    bih = np.asarray(inputs["bih"], np.float32)
    bhh = np.asarray(inputs["bhh"], np.float32)
    shared = dict(
        W1srcT=np.ascontiguousarray(W1[:, :D].T),
        W1tgtT=np.ascontiguousarray(W1[:, D:2 * D].T),
        W1staT=np.ascontiguousarray(np.concatenate(
            [W1[:, 2 * D + 64][None, :], ee @ W1[:, 2 * D:2 * D + 64].T], 0)),
        b1col=np.asarray(inputs["b1"], np.float32)[:, None],
        W2T=np.ascontiguousarray(np.asarray(inputs["W2"], np.float32).T),
        b2col=np.asarray(inputs["b2"], np.float32)[:, None],
        WihT=np.ascontiguousarray(np.asarray(inputs["Wih"], np.float32).T),
        WhhT=np.ascontiguousarray(np.asarray(inputs["Whh"], np.float32).T),
        brzcol=np.ascontiguousarray((bih + bhh)[:2 * D].reshape(2, D).T),
        bnhcol=bhh[2 * D:][:, None].copy(),
        bnicol=bih[2 * D:][:, None].copy(),
        ones1=np.ones((1, D), np.float32),
        eye=np.eye(D, dtype=np.float32),
        iota=np.tile(np.arange(TN, dtype=np.float32), (128, 1)),
    )
    meta = dict(P=P, E_pad=E_pad, n_chunks=n_chunks,
                tile_of_chunk=tile_of_chunk)
    return meta, per_core, shared


def _build(cfg, meta):
    from concourse import bacc, tile, mybir
    import concourse.bass as bass

    nc = bacc.Bacc("TRN2", target_bir_lowering=False, debug=False,
                   num_devices=N_CORES)
    f32, i32, i16 = mybir.dt.float32, mybir.dt.int32, mybir.dt.int16
    AF = mybir.ActivationFunctionType
    NSH, NSHP = cfg.NSH, cfg.NSHP
    P = meta["P"]
    n_chunks = meta["n_chunks"]
    E_pad = meta["E_pad"]

    srcg = nc.dram_tensor("srcg", [128, n_chunks], i32, kind="ExternalInput")
    tlr = nc.dram_tensor("tlr", [128, n_chunks], f32, kind="ExternalInput")
    recg = nc.dram_tensor("recg", [128, n_chunks], f32, kind="ExternalInput")
    apg = nc.dram_tensor("apg", [128, 8 * n_chunks], i16,
                         kind="ExternalInput")
    srhs = nc.dram_tensor("srhs", [5, E_pad], f32, kind="ExternalInput")
    feed = nc.dram_tensor("feed", [E_pad * 128], f32, kind="ExternalInput")
    hsh0T = nc.dram_tensor("hsh0T", [D, NSHP], f32, kind="ExternalInput")
    nomsg = nc.dram_tensor("nomsg", [1, NSHP], f32, kind="ExternalInput")
    wnames = dict(W1srcT=[D, D], W1tgtT=[D, D], W1staT=[5, D],
                  b1col=[D, 1], W2T=[D, D], b2col=[D, 1],
                  WihT=[D, 3 * D], WhhT=[D, 3 * D], brzcol=[D, 2],
                  bnhcol=[D, 1], bnicol=[D, 1], ones1=[1, D], eye=[D, D],
                  iota=[128, TN])
    wt = {k: nc.dram_tensor(k, s, f32, kind="ExternalInput")
          for k, s in wnames.items()}
    h_out = nc.dram_tensor("h_out", [NSHP, D], f32, kind="ExternalOutput")
    h1rm = nc.dram_tensor("h1rm", [NSHP, D], f32)
    h1full = nc.dram_tensor("h1full", [cfg.N, D], f32, addr_space="Shared")

    with tile.TileContext(nc) as tc:
        with (
            tc.tile_pool(name="const", bufs=1) as cpool,
            tc.tile_pool(name="sfeed", bufs=3) as fpool,
            tc.tile_pool(name="work", bufs=3) as wpool,
            tc.tile_pool(name="gru", bufs=1) as upool,
            tc.tile_pool(name="pt", bufs=2, space="PSUM") as pt,
            tc.tile_pool(name="pg", bufs=6, space="PSUM") as pg,
        ):
            w = {}
            for k, s in wnames.items():
                w[k] = cpool.tile(s, f32, tag=k, name=f"w_{k}")
                nc.sync.dma_start(out=w[k][:, :], in_=wt[k][:, :])
            slab = cpool.tile([D, NSHP], f32, tag="slab")
            nc.sync.dma_start(out=slab[:, :], in_=hsh0T[:, :])
            srcg_sb = cpool.tile([128, n_chunks], i32, tag="srcg")
            nc.sync.dma_start(out=srcg_sb[:, :], in_=srcg[:, :])
            tlr_sb = cpool.tile([128, n_chunks], f32, tag="tlr")
            nc.sync.dma_start(out=tlr_sb[:, :], in_=tlr[:, :])
            rec_sb = cpool.tile([128, n_chunks], f32, tag="rec")
            nc.sync.dma_start(out=rec_sb[:, :], in_=recg[:, :])
            apg_sb = cpool.tile([128, 8 * n_chunks], i16, tag="apg")
            nc.sync.dma_start(out=apg_sb[:, :], in_=apg[:, :])

            for p in range(N_PASSES):
                ch0 = 0
                for t in range(cfg.NTILES):
                    tch = int(P[t]) // 128
                    aggT = pg.tile([128, TN], f32, tag="pg", name=f"agg{p}_{t}")
                    for b0 in range(0, tch, 4):
                        bw = min(4, tch - b0)
                        cb = ch0 + b0
                        sT = wpool.tile([128, 512], f32, tag="sT")
                        tT = wpool.tile([128, 512], f32, tag="tT")
                        if p == 0:
                            nc.sync.dma_start(
                                out=sT[:, :128 * bw]
                                .rearrange("p (a e) -> p a e", e=128),
                                in_=feed[cb * 128 * 128:
                                         (cb + bw) * 128 * 128]
                                .rearrange("(a p e) -> p a e", p=128, e=128))
                        for a in range(bw):
                            ch = cb + a
                            if p == 1:
                                gsr = fpool.tile([128, 128], f32, tag="gsr")
                                nc.gpsimd.indirect_dma_start(
                                    out=gsr[:, :], out_offset=None,
                                    in_=h1full[:, :],
                                    in_offset=bass.IndirectOffsetOnAxis(
                                        ap=srcg_sb[:, ch:ch + 1], axis=0))
                                ps = pt.tile([128, 512], f32, tag="pt",
                                             name=f"ps{p}_{ch}")
                                nc.tensor.transpose(
                                    ps[:, :128], gsr[:, :], w["eye"][:, :])
                                nc.vector.tensor_copy(
                                    sT[:, 128 * a:128 * (a + 1)], ps[:, :128])
                            nc.gpsimd.ap_gather(
                                tT[:, 128 * a:128 * (a + 1)],
                                slab[:, TN * t:TN * (t + 1)],
                                apg_sb[:, 8 * ch:8 * (ch + 1)],
                                channels=128, num_elems=TN, d=1, num_idxs=128)
                        y1 = pg.tile([128, 512], f32, tag="pg",
                                     name=f"y1_{p}_{cb}")
                        nc.tensor.matmul(y1[:, :128 * bw], w["W1srcT"][:, :],
                                         sT[:, :128 * bw],
                                         start=True, stop=False)
                        nc.tensor.matmul(y1[:, :128 * bw], w["W1tgtT"][:, :],
                                         tT[:, :128 * bw],
                                         start=False, stop=False)
                        srh_t = fpool.tile([5, 512], f32, tag="srh")
                        nc.sync.dma_start(
                            out=srh_t[:, :128 * bw],
                            in_=srhs[:, 128 * cb:128 * (cb + bw)])
                        nc.tensor.matmul(
                            y1[:, :128 * bw], w["W1staT"][:, :],
                            srh_t[:, :128 * bw],
                            start=False, stop=True)
                        zb = wpool.tile([128, 512], f32, tag="zb")
                        nc.scalar.activation(zb[:, :128 * bw],
                                             y1[:, :128 * bw], AF.Identity,
                                             bias=w["b1col"][:, 0:1])
                        sg = wpool.tile([128, 512], f32, tag="sg")
                        nc.scalar.activation(sg[:, :128 * bw],
                                             y1[:, :128 * bw], AF.Sigmoid,
                                             bias=w["b1col"][:, 0:1])
                        y1s = wpool.tile([128, 512], f32, tag="y1s")
                        nc.vector.tensor_mul(y1s[:, :128 * bw],
                                             zb[:, :128 * bw],
                                             sg[:, :128 * bw])
                        for a in range(bw):
                            ch = cb + a
                            y2 = pt.tile([128, 512], f32, tag="pt",
                                         name=f"y2_{p}_{ch}")
                            nc.tensor.matmul(
                                y2[:, :128], y1s[:, 128 * a:128 * (a + 1)],
                                w["W2T"][:, :], start=True, stop=True)
                            mp = wpool.tile([128, 128], f32, tag="mp")
                            nc.scalar.activation(mp[:, :], y2[:, :128],
                                                 AF.Identity,
                                                 scale=rec_sb[:, ch:ch + 1])
                            S = wpool.tile([128, TN], f32, tag="S")
                            nc.vector.tensor_scalar(
                                out=S[:, :], in0=w["iota"][:, :],
                                scalar1=tlr_sb[:, ch:ch + 1], scalar2=None,
                                op0=mybir.AluOpType.is_equal)
                            nc.tensor.matmul(aggT[:, :], mp[:, :], S[:, :],
                                             start=(b0 == 0 and a == 0),
                                             stop=(b0 + 4 >= tch
                                                   and a == bw - 1))
                    ch0 += tch
                    # ---- GRU for this node tile (in T layout)
                    cl, chh = TN * t, TN * (t + 1)
                    xT = upool.tile([128, TN], f32, tag="xT")
                    nc.scalar.activation(xT[:, :], aggT[:, :], AF.Identity,
                                         bias=w["b2col"][:, 0:1])
                    hTs = slab[:, cl:chh]
                    pr = pg.tile([128, TN], f32, tag="pg", name=f"pr{p}_{t}")
                    pz = pg.tile([128, TN], f32, tag="pg", name=f"pz{p}_{t}")
                    nc.tensor.matmul(pr[:, :], w["WihT"][:, 0:D], xT[:, :],
                                     start=True, stop=False)
                    nc.tensor.matmul(pr[:, :], w["WhhT"][:, 0:D], hTs,
                                     start=False, stop=True)
                    nc.tensor.matmul(pz[:, :], w["WihT"][:, D:2 * D],
                                     xT[:, :], start=True, stop=False)
                    nc.tensor.matmul(pz[:, :], w["WhhT"][:, D:2 * D], hTs,
                                     start=False, stop=False)
                    nm_t = fpool.tile([1, TN], f32, tag="nm")
                    nc.sync.dma_start(out=nm_t[:, :], in_=nomsg[:, cl:chh])
                    nc.tensor.matmul(pz[:, :], w["ones1"][:, :],
                                     nm_t[:, :],
                                     start=False, stop=True)
                    r_s = upool.tile([128, TN], f32, tag="r_s")
                    nc.scalar.activation(r_s[:, :], pr[:, :], AF.Sigmoid,
                                         bias=w["brzcol"][:, 0:1])
                    z_s = upool.tile([128, TN], f32, tag="z_s")
                    nc.scalar.activation(z_s[:, :], pz[:, :], AF.Sigmoid,
                                         bias=w["brzcol"][:, 1:2])
                    pni = pg.tile([128, TN], f32, tag="pg", name=f"pi{p}_{t}")
                    pnh = pg.tile([128, TN], f32, tag="pg", name=f"ph{p}_{t}")
                    nc.tensor.matmul(pni[:, :], w["WihT"][:, 2 * D:3 * D],
                                     xT[:, :], start=True, stop=True)
                    nc.tensor.matmul(pnh[:, :], w["WhhT"][:, 2 * D:3 * D],
                                     hTs, start=True, stop=True)
                    ghn = upool.tile([128, TN], f32, tag="ghn")
                    nc.scalar.activation(ghn[:, :], pnh[:, :], AF.Identity,
                                         bias=w["bnhcol"][:, 0:1])
                    t1 = upool.tile([128, TN], f32, tag="t1")
                    nc.vector.tensor_mul(t1[:, :], r_s[:, :], ghn[:, :])
                    t2 = upool.tile([128, TN], f32, tag="t2")
                    nc.vector.tensor_add(t2[:, :], pni[:, :], t1[:, :])
                    n_s = upool.tile([128, TN], f32, tag="n_s")
                    nc.scalar.activation(n_s[:, :], t2[:, :], AF.Tanh,
                                         bias=w["bnicol"][:, 0:1])
                    d_s = upool.tile([128, TN], f32, tag="d_s")
                    nc.vector.tensor_sub(d_s[:, :], hTs, n_s[:, :])
                    zd = upool.tile([128, TN], f32, tag="zd")
                    nc.vector.tensor_mul(zd[:, :], z_s[:, :], d_s[:, :])
                    hn = upool.tile([128, TN], f32, tag="hn")
                    nc.vector.tensor_add(hn[:, :], n_s[:, :], zd[:, :])
                    # write back into the resident slab (h for next pass)
                    nc.vector.tensor_copy(slab[:, cl:chh], hn[:, :])
                    # back-transpose to row-major for AllGather / output
                    hrows = upool.tile([128, TN], f32, tag="hrows")
                    pb = pt.tile([128, 512], f32, tag="pt", name=f"pb{p}_{t}")
                    for a in range(4):
                        nc.tensor.transpose(
                            pb[:, 128 * a:128 * (a + 1)],
                            hn[:, 128 * a:128 * (a + 1)], w["eye"][:, :])
                        nc.vector.tensor_copy(
                            hrows[:, 128 * a:128 * (a + 1)],
                            pb[:, 128 * a:128 * (a + 1)])
                    dst = h1rm if p == 0 else h_out
                    nc.sync.dma_start(
                        out=dst[cl:chh, :].rearrange("(a q) d -> q a d",
                                                     q=128),
                        in_=hrows[:, :].rearrange("q (a d) -> q a d", d=128))
                if p == 0:
                    nc.gpsimd.collective_compute(
                        "AllGather", mybir.AluOpType.bypass,
                        replica_groups=[list(range(N_CORES))],
                        ins=[h1rm[0:NSH, :]],
                        outs=[h1full[:, :]])
    nc.compile()
    return nc


def build_and_run(inputs, cfg=None, sim=False, trace=False, tmpdir=None):
    cfg = cfg or _Cfg()
    meta, per_core, shared = _plan(cfg, inputs)
    nc = _build(cfg, meta)
    maps = []
    for c in range(N_CORES):
        m = {k: np.ascontiguousarray(v) for k, v in per_core[c].items()}
        m.update({k: np.ascontiguousarray(v) for k, v in shared.items()})
        maps.append(m)
    if sim:
        from concourse.bass_interp import MultiCoreSim
        ms = MultiCoreSim(nc, num_cores=N_CORES, trace=False)
        for c in range(N_CORES):
            for k, v in maps[c].items():
                ms.cores[c].tensor(k)[:] = v
        ms.simulate(check_with_hw=False)
        shards = [np.array(ms.cores[c].tensor("h_out"))[:cfg.NSH]
                  for c in range(N_CORES)]
        return np.concatenate(shards, axis=0), None
    from concourse import bass_utils
    res = bass_utils.run_bass_kernel_spmd(
        nc, maps, list(range(N_CORES)), trace=trace, tmpdir=tmpdir)
    shards = [res.results[c]["h_out"][:cfg.NSH] for c in range(N_CORES)]
    return np.concatenate(shards, axis=0), res


def kernel(**inputs):
    out, _ = build_and_run(inputs)
    return out.astype(np.float32)


# revision 14
# speedup vs baseline: 1.1595x; 1.1595x over previous
"""Trainium2 Bass kernel for BeliefTreeMemory GNN message passing.

Strategy (8 NeuronCores, SPMD, one program):
  - Shard by tgt-node range: core c owns local nodes [0, 25000) = global
    [c*25000, (c+1)*25000).  Edges live on the core owning their tgt.
  - Edges sorted by tgt and grouped by 512-node tiles; per-tile counts
    padded to the max over cores (uniform compile-time structure).
  - h shard kept resident in SBUF, transposed [D, NSH_pad]:
      * tgt-reads  = gpsimd.ap_gather per 128-edge chunk (direct [D,E]).
      * GRU reads/writes the slab in place, tile by tile.
  - Aggregation: indicator matmul  aggT[D, 512] += m'^T @ S  accumulated
    in PSUM across a tile's chunks; S built on DVE by iota==tlocrel.
    recip[tgt] folded into m' (ACT scale); b2 folded into the xT copy.
  - GRU fully in transposed layout; has_msg mask folded as a rank-1
    K=1 matmul adding BIG*(1-has_msg) to the z-gate pre-activation.
  - Pass-1 h_src comes pre-gathered AND pre-transposed from the host
    (h0 is known); pass-2 h_src gathered from the AllGather output via
    per-chunk indirect DMA (128 rows/call).
  - One AllGather of row-major h1 shards between the passes.
"""

import sys
import numpy as np
import ml_dtypes

BF16 = ml_dtypes.bfloat16

sys.path.insert(0, "/opt/trn_rl_repo")

N_NODES = 200000
N_EDGES = 400000
D = 128
N_CORES = 8
N_PASSES = 2
TN = 512            # node-tile (and GRU chunk) size
BIG = 30.0


class _Cfg:
    def __init__(self, n_nodes=N_NODES, n_edges=N_EDGES):
        assert n_nodes % N_CORES == 0
        self.N = n_nodes
        self.E = n_edges
        self.NSH = n_nodes // N_CORES
        self.NSHP = ((self.NSH + TN - 1) // TN) * TN
        self.NTILES = self.NSHP // TN


def _rup(x, m):
    return ((x + m - 1) // m) * m


def _plan(cfg, inputs):
    """Host-side: per-core token layout + all swizzled input images."""
    N, NSH = cfg.N, cfg.NSH
    C = N_CORES
    src = np.asarray(inputs["src"]).astype(np.int64)
    tgt = np.asarray(inputs["tgt"]).astype(np.int64)
    etype = np.asarray(inputs["etype"]).astype(np.int64)
    cred = np.asarray(inputs["cred"], np.float32)
    h0 = np.asarray(inputs["h"], np.float32)
    E = src.shape[0]

    cnt = np.bincount(tgt, minlength=N).astype(np.int64)
    recip = (1.0 / np.maximum(cnt, 1)).astype(np.float32)

    core_of = tgt // NSH
    tloc = tgt - core_of * NSH
    tile_of = tloc // TN

    # per (core, tile) counts -> uniform padded sizes
    counts = np.zeros((C, cfg.NTILES), np.int64)
    for c in range(C):
        m = core_of == c
        counts[c] = np.bincount(tile_of[m], minlength=cfg.NTILES)
    P = np.array([_rup(max(int(counts[:, t].max()), 1), 128)
                  for t in range(cfg.NTILES)], np.int64)
    E_pad = int(P.sum())
    tile_of_chunk = np.repeat(np.arange(cfg.NTILES), P // 128)
    n_chunks = E_pad // 128

    recip_e = recip[tgt]
    cred_e = cred[src]

    per_core = []
    for c in range(C):
        SRC = np.zeros(E_pad, np.int64)
        TLOCREL = np.full(E_pad, -1.0, np.float32)
        TLOC = np.zeros(E_pad, np.int64)       # apg idx (tile-relative)
        REC = np.zeros(E_pad, np.float32)
        SRH = np.zeros((5, E_pad), np.float32)
        m = core_of == c
        eids = np.nonzero(m)[0]
        order = np.argsort(tloc[eids], kind="stable")
        eids = eids[order]
        et = tile_of[eids]
        off = 0
        for t in range(cfg.NTILES):
            ids = eids[et == t]
            nv = len(ids)
            sl = slice(off, off + nv)
            SRC[sl] = src[ids]
            TLOCREL[sl] = (tloc[ids] - t * TN).astype(np.float32)
            TLOC[sl] = tloc[ids] - t * TN
            REC[sl] = recip_e[ids]
            SRH[0, sl] = cred_e[ids]
            oh = np.eye(4, dtype=np.float32)[etype[ids]]
            SRH[1:5, sl] = oh.T
            off += int(P[t])
        assert off == E_pad

        # images
        srcg_img = SRC.reshape(n_chunks, 128).T.astype(np.int32)  # [128, nch]
        tlr_img = TLOCREL.reshape(n_chunks, 128).T.copy()         # [128, nch]
        rec_img = REC.reshape(n_chunks, 128).T.copy()             # [128, nch]
        # apg idx image: per chunk [128, 8]; row p, col s = u[s*16 + p%16]
        apg = np.zeros((128, 8 * n_chunks), np.int16)
        for ch in range(n_chunks):
            u = TLOC[ch * 128:(ch + 1) * 128]
            blk = u.reshape(8, 16).T.astype(np.int16)
            apg[:, 8 * ch:8 * (ch + 1)] = np.tile(blk, (8, 1))
        # pass-1 src feed, pre-transposed per chunk: [nch, 128 D, 128 E]
        feed = h0[SRC].reshape(n_chunks, 128, D).transpose(0, 2, 1)
        feed = np.ascontiguousarray(feed).astype(BF16)

        hshT = np.zeros((D, cfg.NSHP), np.float32)
        hshT[:, :NSH] = h0[c * NSH:(c + 1) * NSH].T
        nomsg = np.zeros((1, cfg.NSHP), np.float32)
        nomsg[0, :NSH] = BIG * (cnt[c * NSH:(c + 1) * NSH] == 0)
        nomsg[0, NSH:] = BIG

        per_core.append(dict(
            srcg=srcg_img, tlr=tlr_img, recg=rec_img, apg=apg,
            srhs=np.ascontiguousarray(SRH.astype(BF16)), feed=feed.reshape(-1),
            hsh0T=hshT, nomsg=nomsg.astype(BF16)))

    W1 = np.asarray(inputs["W1"], np.float32)
    ee = np.asarray(inputs["edge_emb"], np.float32)
    bih = np.asarray(inputs["bih"], np.float32)
    bhh = np.asarray(inputs["bhh"], np.float32)
    shared = dict(
        W1srcT=np.ascontiguousarray(W1[:, :D].T).astype(BF16),
        W1tgtT=np.ascontiguousarray(W1[:, D:2 * D].T).astype(BF16),
        W1staT=np.ascontiguousarray(np.concatenate(
            [W1[:, 2 * D + 64][None, :],
             ee @ W1[:, 2 * D:2 * D + 64].T], 0)).astype(BF16),
        b1col=np.asarray(inputs["b1"], np.float32)[:, None],
        W2T=np.ascontiguousarray(
            np.asarray(inputs["W2"], np.float32).T).astype(BF16),
        b2col=np.asarray(inputs["b2"], np.float32)[:, None],
        WihT=np.ascontiguousarray(
            np.asarray(inputs["Wih"], np.float32).T).astype(BF16),
        WhhT=np.ascontiguousarray(
            np.asarray(inputs["Whh"], np.float32).T).astype(BF16),
        brzcol=np.ascontiguousarray((bih + bhh)[:2 * D].reshape(2, D).T),
        bnhcol=bhh[2 * D:][:, None].copy(),
        bnicol=bih[2 * D:][:, None].copy(),
        ones1=np.ones((1, D), BF16),
        eye=np.eye(D, dtype=np.float32),
        iota=np.tile(np.arange(TN, dtype=np.float32), (128, 1)),
    )
    meta = dict(P=P, E_pad=E_pad, n_chunks=n_chunks,
                tile_of_chunk=tile_of_chunk)
    return meta, per_core, shared


def _build(cfg, meta):
    from concourse import bacc, tile, mybir
    import concourse.bass as bass

    nc = bacc.Bacc("TRN2", target_bir_lowering=False, debug=False,
                   num_devices=N_CORES)
    f32, i32, i16 = mybir.dt.float32, mybir.dt.int32, mybir.dt.int16
    bf16 = mybir.dt.bfloat16
    AF = mybir.ActivationFunctionType
    NSH, NSHP = cfg.NSH, cfg.NSHP
    P = meta["P"]
    n_chunks = meta["n_chunks"]
    E_pad = meta["E_pad"]

    srcg = nc.dram_tensor("srcg", [128, n_chunks], i32, kind="ExternalInput")
    tlr = nc.dram_tensor("tlr", [128, n_chunks], f32, kind="ExternalInput")
    recg = nc.dram_tensor("recg", [128, n_chunks], f32, kind="ExternalInput")
    apg = nc.dram_tensor("apg", [128, 8 * n_chunks], i16,
                         kind="ExternalInput")
    srhs = nc.dram_tensor("srhs", [5, E_pad], bf16, kind="ExternalInput")
    feed = nc.dram_tensor("feed", [E_pad * 128], bf16, kind="ExternalInput")
    hsh0T = nc.dram_tensor("hsh0T", [D, NSHP], f32, kind="ExternalInput")
    nomsg = nc.dram_tensor("nomsg", [1, NSHP], bf16,
                           kind="ExternalInput")
    wnames = dict(W1srcT=[D, D], W1tgtT=[D, D], W1staT=[5, D],
                  b1col=[D, 1], W2T=[D, D], b2col=[D, 1],
                  WihT=[D, 3 * D], WhhT=[D, 3 * D], brzcol=[D, 2],
                  bnhcol=[D, 1], bnicol=[D, 1], ones1=[1, D], eye=[D, D],
                  iota=[128, TN])
    wbf = {"W1srcT", "W1tgtT", "W1staT", "W2T", "WihT", "WhhT", "ones1"}
    wt = {k: nc.dram_tensor(k, sh, bf16 if k in wbf else f32,
                            kind="ExternalInput")
          for k, sh in wnames.items()}
    h_out = nc.dram_tensor("h_out", [NSHP, D], f32, kind="ExternalOutput")
    h1rm = nc.dram_tensor("h1rm", [NSHP, D], f32)
    h1full = nc.dram_tensor("h1full", [cfg.N, D], f32, addr_space="Shared")

    with tile.TileContext(nc) as tc:
        with (
            tc.tile_pool(name="const", bufs=1) as cpool,
            tc.tile_pool(name="sfeed", bufs=3) as fpool,
            tc.tile_pool(name="work", bufs=3) as wpool,
            tc.tile_pool(name="gru", bufs=1) as upool,
            tc.tile_pool(name="pt", bufs=2, space="PSUM") as pt,
            tc.tile_pool(name="pg", bufs=6, space="PSUM") as pg,
        ):
            w = {}
            for k, sh in wnames.items():
                w[k] = cpool.tile(sh, bf16 if k in wbf else f32,
                                  tag=k, name=f"w_{k}")
                nc.sync.dma_start(out=w[k][:, :], in_=wt[k][:, :])
            slab = cpool.tile([D, NSHP], f32, tag="slab")
            for t in range(cfg.NTILES):
                nc.sync.dma_start(out=slab[:, TN * t:TN * (t + 1)],
                                  in_=hsh0T[:, TN * t:TN * (t + 1)])
            srcg_sb = cpool.tile([128, n_chunks], i32, tag="srcg")
            nc.sync.dma_start(out=srcg_sb[:, :], in_=srcg[:, :])
            tlr_sb = cpool.tile([128, n_chunks], f32, tag="tlr")
            nc.sync.dma_start(out=tlr_sb[:, :], in_=tlr[:, :])
            rec_sb = cpool.tile([128, n_chunks], f32, tag="rec")
            nc.sync.dma_start(out=rec_sb[:, :], in_=recg[:, :])
            apg_sb = cpool.tile([128, 8 * n_chunks], i16, tag="apg")
            nc.sync.dma_start(out=apg_sb[:, :], in_=apg[:, :])

            for p in range(N_PASSES):
                ch0 = 0
                for t in range(cfg.NTILES):
                    tch = int(P[t]) // 128
                    aggT = pg.tile([128, TN], f32, tag="pg", name=f"agg{p}_{t}")
                    for b0 in range(0, tch, 4):
                        bw = min(4, tch - b0)
                        cb = ch0 + b0
                        sT = wpool.tile([128, 512], bf16, tag="sT")
                        tT = wpool.tile([128, 512], bf16, tag="tT")
                        if p == 0:
                            nc.sync.dma_start(
                                out=sT[:, :128 * bw]
                                .rearrange("p (a e) -> p a e", e=128),
                                in_=feed[cb * 128 * 128:
                                         (cb + bw) * 128 * 128]
                                .rearrange("(a p e) -> p a e", p=128, e=128))
                        for a in range(bw):
                            ch = cb + a
                            if p == 1:
                                gsr = fpool.tile([128, 128], f32, tag="gsr")
                                nc.gpsimd.indirect_dma_start(
                                    out=gsr[:, :], out_offset=None,
                                    in_=h1full[:, :],
                                    in_offset=bass.IndirectOffsetOnAxis(
                                        ap=srcg_sb[:, ch:ch + 1], axis=0))
                                ps = pt.tile([128, 512], f32, tag="pt",
                                             name=f"ps{p}_{ch}")
                                nc.tensor.transpose(
                                    ps[:, :128], gsr[:, :], w["eye"][:, :])
                                nc.vector.tensor_copy(
                                    sT[:, 128 * a:128 * (a + 1)], ps[:, :128])
                            tTf = fpool.tile([128, 128], f32, tag="tTf")
                            nc.gpsimd.ap_gather(
                                tTf[:, :],
                                slab[:, TN * t:TN * (t + 1)],
                                apg_sb[:, 8 * ch:8 * (ch + 1)],
                                channels=128, num_elems=TN, d=1, num_idxs=128)
                            nc.vector.tensor_copy(
                                tT[:, 128 * a:128 * (a + 1)], tTf[:, :])
                        y1 = pg.tile([128, 512], f32, tag="pg",
                                     name=f"y1_{p}_{cb}")
                        nc.tensor.matmul(y1[:, :128 * bw], w["W1srcT"][:, :],
                                         sT[:, :128 * bw],
                                         start=True, stop=False)
                        nc.tensor.matmul(y1[:, :128 * bw], w["W1tgtT"][:, :],
                                         tT[:, :128 * bw],
                                         start=False, stop=False)
                        srh_t = fpool.tile([5, 512], bf16, tag="srh")
                        nc.sync.dma_start(
                            out=srh_t[:, :128 * bw],
                            in_=srhs[:, 128 * cb:128 * (cb + bw)])
                        nc.tensor.matmul(
                            y1[:, :128 * bw], w["W1staT"][:, :],
                            srh_t[:, :128 * bw],
                            start=False, stop=True)
                        zb = wpool.tile([128, 512], f32, tag="zb")
                        nc.scalar.activation(zb[:, :128 * bw],
                                             y1[:, :128 * bw], AF.Identity,
                                             bias=w["b1col"][:, 0:1])
                        sg = wpool.tile([128, 512], f32, tag="sg")
                        nc.scalar.activation(sg[:, :128 * bw],
                                             y1[:, :128 * bw], AF.Sigmoid,
                                             bias=w["b1col"][:, 0:1])
                        y1s = wpool.tile([128, 512], bf16, tag="y1s")
                        nc.vector.tensor_mul(y1s[:, :128 * bw],
                                             zb[:, :128 * bw],
                                             sg[:, :128 * bw])
                        for a in range(bw):
                            ch = cb + a
                            y2 = pt.tile([128, 512], f32, tag="pt",
                                         name=f"y2_{p}_{ch}")
                            nc.tensor.matmul(
                                y2[:, :128], y1s[:, 128 * a:128 * (a + 1)],
                                w["W2T"][:, :], start=True, stop=True)
                            mp = wpool.tile([128, 128], bf16, tag="mp")
                            nc.scalar.activation(mp[:, :], y2[:, :128],
                                                 AF.Identity,
                                                 scale=rec_sb[:, ch:ch + 1])
                            S = wpool.tile([128, TN], bf16, tag="S")
                            nc.vector.tensor_scalar(
                                out=S[:, :], in0=w["iota"][:, :],
                                scalar1=tlr_sb[:, ch:ch + 1], scalar2=None,
                                op0=mybir.AluOpType.is_equal)
                            nc.tensor.matmul(aggT[:, :], mp[:, :], S[:, :],
                                             start=(b0 == 0 and a == 0),
                                             stop=(b0 + 4 >= tch
                                                   and a == bw - 1))
                    ch0 += tch
                    # ---- GRU for this node tile (in T layout)
                    cl, chh = TN * t, TN * (t + 1)
                    xT = upool.tile([128, TN], bf16, tag="xT")
                    nc.scalar.activation(xT[:, :], aggT[:, :], AF.Identity,
                                         bias=w["b2col"][:, 0:1])
                    hTs = slab[:, cl:chh]
                    hTb = upool.tile([128, TN], bf16, tag="hTb")
                    nc.vector.tensor_copy(hTb[:, :], hTs)
                    pr = pg.tile([128, TN], f32, tag="pg", name=f"pr{p}_{t}")
                    pz = pg.tile([128, TN], f32, tag="pg", name=f"pz{p}_{t}")
                    nc.tensor.matmul(pr[:, :], w["WihT"][:, 0:D], xT[:, :],
                                     start=True, stop=False)
                    nc.tensor.matmul(pr[:, :], w["WhhT"][:, 0:D], hTb[:, :],
                                     start=False, stop=True)
                    nc.tensor.matmul(pz[:, :], w["WihT"][:, D:2 * D],
                                     xT[:, :], start=True, stop=False)
                    nc.tensor.matmul(pz[:, :], w["WhhT"][:, D:2 * D], hTb[:, :],
                                     start=False, stop=False)
                    nm_t = fpool.tile([1, TN], bf16, tag="nm")
                    nc.sync.dma_start(out=nm_t[:, :], in_=nomsg[:, cl:chh])
                    nc.tensor.matmul(pz[:, :], w["ones1"][:, :],
                                     nm_t[:, :], start=False, stop=True)
                    r_s = upool.tile([128, TN], f32, tag="r_s")
                    nc.scalar.activation(r_s[:, :], pr[:, :], AF.Sigmoid,
                                         bias=w["brzcol"][:, 0:1])
                    z_s = upool.tile([128, TN], f32, tag="z_s")
                    nc.scalar.activation(z_s[:, :], pz[:, :], AF.Sigmoid,
                                         bias=w["brzcol"][:, 1:2])
                    pni = pg.tile([128, TN], f32, tag="pg", name=f"pi{p}_{t}")
                    pnh = pg.tile([128, TN], f32, tag="pg", name=f"ph{p}_{t}")
                    nc.tensor.matmul(pni[:, :], w["WihT"][:, 2 * D:3 * D],
                                     xT[:, :], start=True, stop=True)
                    nc.tensor.matmul(pnh[:, :], w["WhhT"][:, 2 * D:3 * D],
                                     hTb[:, :], start=True, stop=True)
                    ghn = upool.tile([128, TN], f32, tag="ghn")
                    nc.scalar.activation(ghn[:, :], pnh[:, :], AF.Identity,
                                         bias=w["bnhcol"][:, 0:1])
                    t1 = upool.tile([128, TN], f32, tag="t1")
                    nc.vector.tensor_mul(t1[:, :], r_s[:, :], ghn[:, :])
                    t2 = upool.tile([128, TN], f32, tag="t2")
                    nc.vector.tensor_add(t2[:, :], pni[:, :], t1[:, :])
                    n_s = upool.tile([128, TN], f32, tag="n_s")
                    nc.scalar.activation(n_s[:, :], t2[:, :], AF.Tanh,
                                         bias=w["bnicol"][:, 0:1])
                    d_s = upool.tile([128, TN], f32, tag="d_s")
                    nc.vector.tensor_sub(d_s[:, :], hTs, n_s[:, :])
                    zd = upool.tile([128, TN], f32, tag="zd")
                    nc.vector.tensor_mul(zd[:, :], z_s[:, :], d_s[:, :])
                    hn = upool.tile([128, TN], f32, tag="hn")
                    nc.vector.tensor_add(hn[:, :], n_s[:, :], zd[:, :])
                    # write back into the resident slab (h for next pass)
                    nc.vector.tensor_copy(slab[:, cl:chh], hn[:, :])
                    # back-transpose to row-major for AllGather / output
                    hrows = upool.tile([128, TN], f32, tag="hrows")
                    pb = pt.tile([128, 512], f32, tag="pt", name=f"pb{p}_{t}")
                    for a in range(4):
                        nc.tensor.transpose(
                            pb[:, 128 * a:128 * (a + 1)],
                            hn[:, 128 * a:128 * (a + 1)], w["eye"][:, :])
                        nc.vector.tensor_copy(
                            hrows[:, 128 * a:128 * (a + 1)],
                            pb[:, 128 * a:128 * (a + 1)])
                    dst = h1rm if p == 0 else h_out
                    nc.sync.dma_start(
                        out=dst[cl:chh, :].rearrange("(a q) d -> q a d",
                                                     q=128),
                        in_=hrows[:, :].rearrange("q (a d) -> q a d", d=128))
                if p == 0:
                    nc.gpsimd.collective_compute(
                        "AllGather", mybir.AluOpType.bypass,
                        replica_groups=[list(range(N_CORES))],
                        ins=[h1rm[0:NSH, :]],
                        outs=[h1full[:, :]])
    nc.compile()
    return nc


def build_and_run(inputs, cfg=None, sim=False, trace=False, tmpdir=None):
    cfg = cfg or _Cfg()
    meta, per_core, shared = _plan(cfg, inputs)
    nc = _build(cfg, meta)
    maps = []
    for c in range(N_CORES):
        m = {k: np.ascontiguousarray(v) for k, v in per_core[c].items()}
        m.update({k: np.ascontiguousarray(v) for k, v in shared.items()})
        maps.append(m)
    if sim:
        from concourse.bass_interp import MultiCoreSim
        ms = MultiCoreSim(nc, num_cores=N_CORES, trace=False)
        for c in range(N_CORES):
            for k, v in maps[c].items():
                ms.cores[c].tensor(k)[:] = v
        ms.simulate(check_with_hw=False)
        shards = [np.array(ms.cores[c].tensor("h_out"))[:cfg.NSH]
                  for c in range(N_CORES)]
        return np.concatenate(shards, axis=0), None
    from concourse import bass_utils
    res = bass_utils.run_bass_kernel_spmd(
        nc, maps, list(range(N_CORES)), trace=trace, tmpdir=tmpdir)
    shards = [res.results[c]["h_out"][:cfg.NSH] for c in range(N_CORES)]
    return np.concatenate(shards, axis=0), res


def kernel(**inputs):
    out, _ = build_and_run(inputs)
    return out.astype(np.float32)


# revision 15
# speedup vs baseline: 1.1632x; 1.0032x over previous
"""Trainium2 Bass kernel for BeliefTreeMemory GNN message passing.

Strategy (8 NeuronCores, SPMD, one program):
  - Shard by tgt-node range: core c owns local nodes [0, 25000) = global
    [c*25000, (c+1)*25000).  Edges live on the core owning their tgt.
  - Edges sorted by tgt and grouped by 512-node tiles; per-tile counts
    padded to the max over cores (uniform compile-time structure).
  - h shard kept resident in SBUF, transposed [D, NSH_pad]:
      * tgt-reads  = gpsimd.ap_gather per 128-edge chunk (direct [D,E]).
      * GRU reads/writes the slab in place, tile by tile.
  - Aggregation: indicator matmul  aggT[D, 512] += m'^T @ S  accumulated
    in PSUM across a tile's chunks; S built on DVE by iota==tlocrel.
    recip[tgt] folded into m' (ACT scale); b2 folded into the xT copy.
  - GRU fully in transposed layout; has_msg mask folded as a rank-1
    K=1 matmul adding BIG*(1-has_msg) to the z-gate pre-activation.
  - Pass-1 h_src comes pre-gathered AND pre-transposed from the host
    (h0 is known); pass-2 h_src gathered from the AllGather output via
    per-chunk indirect DMA (128 rows/call).
  - One AllGather of row-major h1 shards between the passes.
"""

import sys
import numpy as np
import ml_dtypes

BF16 = ml_dtypes.bfloat16

sys.path.insert(0, "/opt/trn_rl_repo")

N_NODES = 200000
N_EDGES = 400000
D = 128
N_CORES = 8
N_PASSES = 2
TN = 512            # node-tile (and GRU chunk) size
FAKE_SILU = False   # decomposed silu (sim lacks Silu table)
BIG = 30.0


class _Cfg:
    def __init__(self, n_nodes=N_NODES, n_edges=N_EDGES):
        assert n_nodes % N_CORES == 0
        self.N = n_nodes
        self.E = n_edges
        self.NSH = n_nodes // N_CORES
        self.NSHP = ((self.NSH + TN - 1) // TN) * TN
        self.NTILES = self.NSHP // TN


def _rup(x, m):
    return ((x + m - 1) // m) * m


def _plan(cfg, inputs):
    """Host-side: per-core token layout + all swizzled input images."""
    N, NSH = cfg.N, cfg.NSH
    C = N_CORES
    src = np.asarray(inputs["src"]).astype(np.int64)
    tgt = np.asarray(inputs["tgt"]).astype(np.int64)
    etype = np.asarray(inputs["etype"]).astype(np.int64)
    cred = np.asarray(inputs["cred"], np.float32)
    h0 = np.asarray(inputs["h"], np.float32)
    E = src.shape[0]

    cnt = np.bincount(tgt, minlength=N).astype(np.int64)
    recip = (1.0 / np.maximum(cnt, 1)).astype(np.float32)

    core_of = tgt // NSH
    tloc = tgt - core_of * NSH
    tile_of = tloc // TN

    # per (core, tile) counts -> uniform padded sizes
    counts = np.zeros((C, cfg.NTILES), np.int64)
    for c in range(C):
        m = core_of == c
        counts[c] = np.bincount(tile_of[m], minlength=cfg.NTILES)
    P = np.array([_rup(max(int(counts[:, t].max()), 1), 128)
                  for t in range(cfg.NTILES)], np.int64)
    E_pad = int(P.sum())
    tile_of_chunk = np.repeat(np.arange(cfg.NTILES), P // 128)
    n_chunks = E_pad // 128

    recip_e = recip[tgt]
    cred_e = cred[src]

    per_core = []
    for c in range(C):
        SRC = np.zeros(E_pad, np.int64)
        TLOCREL = np.full(E_pad, -1.0, np.float32)
        TLOC = np.zeros(E_pad, np.int64)       # apg idx (tile-relative)
        REC = np.zeros(E_pad, np.float32)
        SRH = np.zeros((5, E_pad), np.float32)
        m = core_of == c
        eids = np.nonzero(m)[0]
        order = np.argsort(tloc[eids], kind="stable")
        eids = eids[order]
        et = tile_of[eids]
        off = 0
        for t in range(cfg.NTILES):
            ids = eids[et == t]
            nv = len(ids)
            sl = slice(off, off + nv)
            SRC[sl] = src[ids]
            TLOCREL[sl] = (tloc[ids] - t * TN).astype(np.float32)
            TLOC[sl] = tloc[ids] - t * TN
            REC[sl] = recip_e[ids]
            SRH[0, sl] = cred_e[ids]
            oh = np.eye(4, dtype=np.float32)[etype[ids]]
            SRH[1:5, sl] = oh.T
            off += int(P[t])
        assert off == E_pad

        # images
        srcg_img = SRC.reshape(n_chunks, 128).T.astype(np.int32)  # [128, nch]
        tlr_img = TLOCREL.reshape(n_chunks, 128).T.copy()         # [128, nch]
        rec_img = REC.reshape(n_chunks, 128).T.copy()             # [128, nch]
        # apg idx image: per chunk [128, 8]; row p, col s = u[s*16 + p%16]
        apg = np.zeros((128, 8 * n_chunks), np.int16)
        for ch in range(n_chunks):
            u = TLOC[ch * 128:(ch + 1) * 128]
            blk = u.reshape(8, 16).T.astype(np.int16)
            apg[:, 8 * ch:8 * (ch + 1)] = np.tile(blk, (8, 1))
        # pass-1 src feed, pre-transposed per chunk: [nch, 128 D, 128 E]
        feed = h0[SRC].reshape(n_chunks, 128, D).transpose(0, 2, 1)
        feed = np.ascontiguousarray(feed).astype(BF16)

        hshT = np.zeros((D, cfg.NSHP), np.float32)
        hshT[:, :NSH] = h0[c * NSH:(c + 1) * NSH].T
        nomsg = np.zeros((1, cfg.NSHP), np.float32)
        nomsg[0, :NSH] = BIG * (cnt[c * NSH:(c + 1) * NSH] == 0)
        nomsg[0, NSH:] = BIG

        per_core.append(dict(
            srcg=srcg_img, tlr=tlr_img, recg=rec_img, apg=apg,
            srhs=np.ascontiguousarray(SRH.astype(BF16)), feed=feed.reshape(-1),
            hsh0T=hshT, nomsg=nomsg.astype(BF16)))

    W1 = np.asarray(inputs["W1"], np.float32)
    ee = np.asarray(inputs["edge_emb"], np.float32)
    bih = np.asarray(inputs["bih"], np.float32)
    bhh = np.asarray(inputs["bhh"], np.float32)
    shared = dict(
        W1srcT=np.ascontiguousarray(W1[:, :D].T).astype(BF16),
        W1tgtT=np.ascontiguousarray(W1[:, D:2 * D].T).astype(BF16),
        W1staT=np.ascontiguousarray(np.concatenate(
            [W1[:, 2 * D + 64][None, :],
             ee @ W1[:, 2 * D:2 * D + 64].T], 0)).astype(BF16),
        b1col=np.asarray(inputs["b1"], np.float32)[:, None],
        W2T=np.ascontiguousarray(
            np.asarray(inputs["W2"], np.float32).T).astype(BF16),
        b2col=np.asarray(inputs["b2"], np.float32)[:, None],
        WihT=np.ascontiguousarray(
            np.asarray(inputs["Wih"], np.float32).T).astype(BF16),
        WhhT=np.ascontiguousarray(
            np.asarray(inputs["Whh"], np.float32).T).astype(BF16),
        brzcol=np.ascontiguousarray((bih + bhh)[:2 * D].reshape(2, D).T),
        bnhcol=bhh[2 * D:][:, None].copy(),
        bnicol=bih[2 * D:][:, None].copy(),
        ones1=np.ones((1, D), BF16),
        eye=np.eye(D, dtype=np.float32),
        iota=np.tile(np.arange(TN, dtype=np.float32), (128, 1)),
    )
    meta = dict(P=P, E_pad=E_pad, n_chunks=n_chunks,
                tile_of_chunk=tile_of_chunk)
    return meta, per_core, shared


def _build(cfg, meta):
    global FAKE_SILU
    from concourse import bacc, tile, mybir
    import concourse.bass as bass

    nc = bacc.Bacc("TRN2", target_bir_lowering=False, debug=False,
                   num_devices=N_CORES)
    f32, i32, i16 = mybir.dt.float32, mybir.dt.int32, mybir.dt.int16
    bf16 = mybir.dt.bfloat16
    AF = mybir.ActivationFunctionType
    NSH, NSHP = cfg.NSH, cfg.NSHP
    P = meta["P"]
    n_chunks = meta["n_chunks"]
    E_pad = meta["E_pad"]

    srcg = nc.dram_tensor("srcg", [128, n_chunks], i32, kind="ExternalInput")
    tlr = nc.dram_tensor("tlr", [128, n_chunks], f32, kind="ExternalInput")
    recg = nc.dram_tensor("recg", [128, n_chunks], f32, kind="ExternalInput")
    apg = nc.dram_tensor("apg", [128, 8 * n_chunks], i16,
                         kind="ExternalInput")
    srhs = nc.dram_tensor("srhs", [5, E_pad], bf16, kind="ExternalInput")
    feed = nc.dram_tensor("feed", [E_pad * 128], bf16, kind="ExternalInput")
    hsh0T = nc.dram_tensor("hsh0T", [D, NSHP], f32, kind="ExternalInput")
    nomsg = nc.dram_tensor("nomsg", [1, NSHP], bf16,
                           kind="ExternalInput")
    wnames = dict(W1srcT=[D, D], W1tgtT=[D, D], W1staT=[5, D],
                  b1col=[D, 1], W2T=[D, D], b2col=[D, 1],
                  WihT=[D, 3 * D], WhhT=[D, 3 * D], brzcol=[D, 2],
                  bnhcol=[D, 1], bnicol=[D, 1], ones1=[1, D], eye=[D, D],
                  iota=[128, TN])
    wbf = {"W1srcT", "W1tgtT", "W1staT", "W2T", "WihT", "WhhT", "ones1"}
    wt = {k: nc.dram_tensor(k, sh, bf16 if k in wbf else f32,
                            kind="ExternalInput")
          for k, sh in wnames.items()}
    h_out = nc.dram_tensor("h_out", [NSHP, D], f32, kind="ExternalOutput")
    h1rm = nc.dram_tensor("h1rm", [NSHP, D], f32)
    h1full = nc.dram_tensor("h1full", [cfg.N, D], f32, addr_space="Shared")

    with tile.TileContext(nc) as tc:
        with (
            tc.tile_pool(name="const", bufs=1) as cpool,
            tc.tile_pool(name="sfeed", bufs=3) as fpool,
            tc.tile_pool(name="work", bufs=3) as wpool,
            tc.tile_pool(name="gru", bufs=1) as upool,
            tc.tile_pool(name="pt", bufs=2, space="PSUM") as pt,
            tc.tile_pool(name="pg", bufs=4, space="PSUM") as pg,
            tc.tile_pool(name="pu", bufs=2, space="PSUM") as pu,
        ):
            w = {}
            for k, sh in wnames.items():
                w[k] = cpool.tile(sh, bf16 if k in wbf else f32,
                                  tag=k, name=f"w_{k}")
                nc.sync.dma_start(out=w[k][:, :], in_=wt[k][:, :])
            slab = cpool.tile([D, NSHP], f32, tag="slab")
            for t in range(cfg.NTILES):
                nc.sync.dma_start(out=slab[:, TN * t:TN * (t + 1)],
                                  in_=hsh0T[:, TN * t:TN * (t + 1)])
            srcg_sb = cpool.tile([128, n_chunks], i32, tag="srcg")
            nc.sync.dma_start(out=srcg_sb[:, :], in_=srcg[:, :])
            tlr_sb = cpool.tile([128, n_chunks], f32, tag="tlr")
            nc.sync.dma_start(out=tlr_sb[:, :], in_=tlr[:, :])
            rec_sb = cpool.tile([128, n_chunks], f32, tag="rec")
            nc.sync.dma_start(out=rec_sb[:, :], in_=recg[:, :])
            apg_sb = cpool.tile([128, 8 * n_chunks], i16, tag="apg")
            nc.sync.dma_start(out=apg_sb[:, :], in_=apg[:, :])

            for p in range(N_PASSES):
                ch0 = 0
                for t in range(cfg.NTILES):
                    tch = int(P[t]) // 128
                    aggT = pg.tile([128, TN], f32, tag="pg", name=f"agg{p}_{t}")
                    for b0 in range(0, tch, 4):
                        bw = min(4, tch - b0)
                        cb = ch0 + b0
                        sT = wpool.tile([128, 512], bf16, tag="sT")
                        tT = wpool.tile([128, 512], bf16, tag="tT")
                        if p == 0:
                            nc.sync.dma_start(
                                out=sT[:, :128 * bw]
                                .rearrange("p (a e) -> p a e", e=128),
                                in_=feed[cb * 128 * 128:
                                         (cb + bw) * 128 * 128]
                                .rearrange("(a p e) -> p a e", p=128, e=128))
                        for a in range(bw):
                            ch = cb + a
                            if p == 1:
                                gsr = fpool.tile([128, 128], f32, tag="gsr")
                                nc.gpsimd.indirect_dma_start(
                                    out=gsr[:, :], out_offset=None,
                                    in_=h1full[:, :],
                                    in_offset=bass.IndirectOffsetOnAxis(
                                        ap=srcg_sb[:, ch:ch + 1], axis=0))
                                ps = pt.tile([128, 512], f32, tag="pt",
                                             name=f"ps{p}_{ch}")
                                nc.tensor.transpose(
                                    ps[:, :128], gsr[:, :], w["eye"][:, :])
                                nc.vector.tensor_copy(
                                    sT[:, 128 * a:128 * (a + 1)], ps[:, :128])
                            tTf = fpool.tile([128, 128], f32, tag="tTf")
                            nc.gpsimd.ap_gather(
                                tTf[:, :],
                                slab[:, TN * t:TN * (t + 1)],
                                apg_sb[:, 8 * ch:8 * (ch + 1)],
                                channels=128, num_elems=TN, d=1, num_idxs=128)
                            nc.vector.tensor_copy(
                                tT[:, 128 * a:128 * (a + 1)], tTf[:, :])
                        y1 = pg.tile([128, 512], f32, tag="pg",
                                     name=f"y1_{p}_{cb}")
                        nc.tensor.matmul(y1[:, :128 * bw], w["W1srcT"][:, :],
                                         sT[:, :128 * bw],
                                         start=True, stop=False)
                        nc.tensor.matmul(y1[:, :128 * bw], w["W1tgtT"][:, :],
                                         tT[:, :128 * bw],
                                         start=False, stop=False)
                        srh_t = fpool.tile([5, 512], bf16, tag="srh")
                        nc.sync.dma_start(
                            out=srh_t[:, :128 * bw],
                            in_=srhs[:, 128 * cb:128 * (cb + bw)])
                        nc.tensor.matmul(
                            y1[:, :128 * bw], w["W1staT"][:, :],
                            srh_t[:, :128 * bw],
                            start=False, stop=True)
                        y1s = wpool.tile([128, 512], bf16, tag="y1s")
                        if FAKE_SILU:
                            zb = wpool.tile([128, 512], f32, tag="zb")
                            nc.scalar.activation(zb[:, :128 * bw],
                                                 y1[:, :128 * bw],
                                                 AF.Identity,
                                                 bias=w["b1col"][:, 0:1])
                            sg = wpool.tile([128, 512], f32, tag="sg")
                            nc.scalar.activation(sg[:, :128 * bw],
                                                 y1[:, :128 * bw],
                                                 AF.Sigmoid,
                                                 bias=w["b1col"][:, 0:1])
                            nc.vector.tensor_mul(y1s[:, :128 * bw],
                                                 zb[:, :128 * bw],
                                                 sg[:, :128 * bw])
                        else:
                            nc.scalar.activation(y1s[:, :128 * bw],
                                                 y1[:, :128 * bw], AF.Silu,
                                                 bias=w["b1col"][:, 0:1])
                        for a in range(bw):
                            ch = cb + a
                            y2 = pt.tile([128, 512], f32, tag="pt",
                                         name=f"y2_{p}_{ch}")
                            nc.tensor.matmul(
                                y2[:, :128], y1s[:, 128 * a:128 * (a + 1)],
                                w["W2T"][:, :], start=True, stop=True)
                            mp = wpool.tile([128, 128], bf16, tag="mp")
                            nc.scalar.activation(mp[:, :], y2[:, :128],
                                                 AF.Identity,
                                                 scale=rec_sb[:, ch:ch + 1])
                            S = wpool.tile([128, TN], bf16, tag="S")
                            nc.vector.tensor_scalar(
                                out=S[:, :], in0=w["iota"][:, :],
                                scalar1=tlr_sb[:, ch:ch + 1], scalar2=None,
                                op0=mybir.AluOpType.is_equal)
                            nc.tensor.matmul(aggT[:, :], mp[:, :], S[:, :],
                                             start=(b0 == 0 and a == 0),
                                             stop=(b0 + 4 >= tch
                                                   and a == bw - 1))
                    ch0 += tch
                    # ---- GRU for this node tile (in T layout)
                    cl, chh = TN * t, TN * (t + 1)
                    xT = upool.tile([128, TN], bf16, tag="xT")
                    nc.scalar.activation(xT[:, :], aggT[:, :], AF.Identity,
                                         bias=w["b2col"][:, 0:1])
                    hTs = slab[:, cl:chh]
                    hTb = upool.tile([128, TN], bf16, tag="hTb")
                    nc.vector.tensor_copy(hTb[:, :], hTs)
                    pr = pu.tile([128, TN], f32, tag="pu", name=f"pr{p}_{t}")
                    pz = pu.tile([128, TN], f32, tag="pu", name=f"pz{p}_{t}")
                    nc.tensor.matmul(pr[:, :], w["WihT"][:, 0:D], xT[:, :],
                                     start=True, stop=False)
                    nc.tensor.matmul(pr[:, :], w["WhhT"][:, 0:D], hTb[:, :],
                                     start=False, stop=True)
                    nc.tensor.matmul(pz[:, :], w["WihT"][:, D:2 * D],
                                     xT[:, :], start=True, stop=False)
                    nc.tensor.matmul(pz[:, :], w["WhhT"][:, D:2 * D], hTb[:, :],
                                     start=False, stop=False)
                    nm_t = fpool.tile([1, TN], bf16, tag="nm")
                    nc.sync.dma_start(out=nm_t[:, :], in_=nomsg[:, cl:chh])
                    nc.tensor.matmul(pz[:, :], w["ones1"][:, :],
                                     nm_t[:, :], start=False, stop=True)
                    r_s = upool.tile([128, TN], f32, tag="r_s")
                    nc.scalar.activation(r_s[:, :], pr[:, :], AF.Sigmoid,
                                         bias=w["brzcol"][:, 0:1])
                    z_s = upool.tile([128, TN], f32, tag="z_s")
                    nc.scalar.activation(z_s[:, :], pz[:, :], AF.Sigmoid,
                                         bias=w["brzcol"][:, 1:2])
                    pni = pu.tile([128, TN], f32, tag="pu", name=f"pi{p}_{t}")
                    pnh = pu.tile([128, TN], f32, tag="pu", name=f"ph{p}_{t}")
                    nc.tensor.matmul(pni[:, :], w["WihT"][:, 2 * D:3 * D],
                                     xT[:, :], start=True, stop=True)
                    nc.tensor.matmul(pnh[:, :], w["WhhT"][:, 2 * D:3 * D],
                                     hTb[:, :], start=True, stop=True)
                    ghn = upool.tile([128, TN], f32, tag="ghn")
                    nc.scalar.activation(ghn[:, :], pnh[:, :], AF.Identity,
                                         bias=w["bnhcol"][:, 0:1])
                    t1 = upool.tile([128, TN], f32, tag="t1")
                    nc.vector.tensor_mul(t1[:, :], r_s[:, :], ghn[:, :])
                    t2 = upool.tile([128, TN], f32, tag="t2")
                    nc.vector.tensor_add(t2[:, :], pni[:, :], t1[:, :])
                    n_s = upool.tile([128, TN], f32, tag="n_s")
                    nc.scalar.activation(n_s[:, :], t2[:, :], AF.Tanh,
                                         bias=w["bnicol"][:, 0:1])
                    d_s = upool.tile([128, TN], f32, tag="d_s")
                    nc.vector.tensor_sub(d_s[:, :], hTs, n_s[:, :])
                    zd = upool.tile([128, TN], f32, tag="zd")
                    nc.vector.tensor_mul(zd[:, :], z_s[:, :], d_s[:, :])
                    hn = upool.tile([128, TN], f32, tag="hn")
                    nc.vector.tensor_add(hn[:, :], n_s[:, :], zd[:, :])
                    # write back into the resident slab (h for next pass)
                    nc.vector.tensor_copy(slab[:, cl:chh], hn[:, :])
                    # back-transpose to row-major for AllGather / output
                    hrows = upool.tile([128, TN], f32, tag="hrows")
                    pb = pt.tile([128, 512], f32, tag="pt", name=f"pb{p}_{t}")
                    for a in range(4):
                        nc.tensor.transpose(
                            pb[:, 128 * a:128 * (a + 1)],
                            hn[:, 128 * a:128 * (a + 1)], w["eye"][:, :])
                        nc.vector.tensor_copy(
                            hrows[:, 128 * a:128 * (a + 1)],
                            pb[:, 128 * a:128 * (a + 1)])
                    dst = h1rm if p == 0 else h_out
                    nc.sync.dma_start(
                        out=dst[cl:chh, :].rearrange("(a q) d -> q a d",
                                                     q=128),
                        in_=hrows[:, :].rearrange("q (a d) -> q a d", d=128))
                if p == 0:
                    nc.gpsimd.collective_compute(
                        "AllGather", mybir.AluOpType.bypass,
                        replica_groups=[list(range(N_CORES))],
                        ins=[h1rm[0:NSH, :]],
                        outs=[h1full[:, :]])
    nc.compile()
    return nc


def build_and_run(inputs, cfg=None, sim=False, trace=False, tmpdir=None):
    global FAKE_SILU
    cfg = cfg or _Cfg()
    meta, per_core, shared = _plan(cfg, inputs)
    FAKE_SILU = bool(sim)
    nc = _build(cfg, meta)
    maps = []
    for c in range(N_CORES):
        m = {k: np.ascontiguousarray(v) for k, v in per_core[c].items()}
        m.update({k: np.ascontiguousarray(v) for k, v in shared.items()})
        maps.append(m)
    if sim:
        from concourse.bass_interp import MultiCoreSim
        ms = MultiCoreSim(nc, num_cores=N_CORES, trace=False)
        for c in range(N_CORES):
            for k, v in maps[c].items():
                ms.cores[c].tensor(k)[:] = v
        ms.simulate(check_with_hw=False)
        shards = [np.array(ms.cores[c].tensor("h_out"))[:cfg.NSH]
                  for c in range(N_CORES)]
        return np.concatenate(shards, axis=0), None
    from concourse import bass_utils
    res = bass_utils.run_bass_kernel_spmd(
        nc, maps, list(range(N_CORES)), trace=trace, tmpdir=tmpdir)
    shards = [res.results[c]["h_out"][:cfg.NSH] for c in range(N_CORES)]
    return np.concatenate(shards, axis=0), res


def kernel(**inputs):
    out, _ = build_and_run(inputs)
    return out.astype(np.float32)


# revision 16
# speedup vs baseline: 1.1792x; 1.0137x over previous
"""Trainium2 Bass kernel for BeliefTreeMemory GNN message passing.

Strategy (8 NeuronCores, SPMD, one program):
  - Shard by tgt-node range: core c owns local nodes [0, 25000) = global
    [c*25000, (c+1)*25000).  Edges live on the core owning their tgt.
  - Edges sorted by tgt and grouped by 512-node tiles; per-tile counts
    padded to the max over cores (uniform compile-time structure).
  - h shard kept resident in SBUF, transposed [D, NSH_pad]:
      * tgt-reads  = gpsimd.ap_gather per 128-edge chunk (direct [D,E]).
      * GRU reads/writes the slab in place, tile by tile.
  - Aggregation: indicator matmul  aggT[D, 512] += m'^T @ S  accumulated
    in PSUM across a tile's chunks; S built on DVE by iota==tlocrel.
    recip[tgt] folded into m' (ACT scale); b2 folded into the xT copy.
  - GRU fully in transposed layout; has_msg mask folded as a rank-1
    K=1 matmul adding BIG*(1-has_msg) to the z-gate pre-activation.
  - Pass-1 h_src comes pre-gathered AND pre-transposed from the host
    (h0 is known); pass-2 h_src gathered from the AllGather output via
    per-chunk indirect DMA (128 rows/call).
  - One AllGather of row-major h1 shards between the passes.
"""

import sys
import numpy as np
import ml_dtypes

BF16 = ml_dtypes.bfloat16

sys.path.insert(0, "/opt/trn_rl_repo")

N_NODES = 200000
N_EDGES = 400000
D = 128
N_CORES = 8
N_PASSES = 2
TN = 512            # node-tile (and GRU chunk) size
FAKE_SILU = False   # decomposed silu (sim lacks Silu table)
BIG = 30.0


class _Cfg:
    def __init__(self, n_nodes=N_NODES, n_edges=N_EDGES):
        assert n_nodes % N_CORES == 0
        self.N = n_nodes
        self.E = n_edges
        self.NSH = n_nodes // N_CORES
        self.NSHP = ((self.NSH + TN - 1) // TN) * TN
        self.NTILES = self.NSHP // TN


def _rup(x, m):
    return ((x + m - 1) // m) * m


def _plan(cfg, inputs):
    """Host-side: per-core token layout + all swizzled input images."""
    N, NSH = cfg.N, cfg.NSH
    C = N_CORES
    src = np.asarray(inputs["src"]).astype(np.int64)
    tgt = np.asarray(inputs["tgt"]).astype(np.int64)
    etype = np.asarray(inputs["etype"]).astype(np.int64)
    cred = np.asarray(inputs["cred"], np.float32)
    h0 = np.asarray(inputs["h"], np.float32)
    E = src.shape[0]

    cnt = np.bincount(tgt, minlength=N).astype(np.int64)
    recip = (1.0 / np.maximum(cnt, 1)).astype(np.float32)

    core_of = tgt // NSH
    tloc = tgt - core_of * NSH
    tile_of = tloc // TN

    # per (core, tile) counts -> uniform padded sizes
    counts = np.zeros((C, cfg.NTILES), np.int64)
    for c in range(C):
        m = core_of == c
        counts[c] = np.bincount(tile_of[m], minlength=cfg.NTILES)
    P = np.array([_rup(max(int(counts[:, t].max()), 1), 128)
                  for t in range(cfg.NTILES)], np.int64)
    E_pad = int(P.sum())
    tile_of_chunk = np.repeat(np.arange(cfg.NTILES), P // 128)
    n_chunks = E_pad // 128

    recip_e = recip[tgt]
    cred_e = cred[src]

    per_core = []
    for c in range(C):
        SRC = np.zeros(E_pad, np.int64)
        TLOCREL = np.full(E_pad, -1.0, np.float32)
        TLOC = np.zeros(E_pad, np.int64)       # apg idx (tile-relative)
        REC = np.zeros(E_pad, np.float32)
        SRH = np.zeros((5, E_pad), np.float32)
        m = core_of == c
        eids = np.nonzero(m)[0]
        order = np.argsort(tloc[eids], kind="stable")
        eids = eids[order]
        et = tile_of[eids]
        off = 0
        for t in range(cfg.NTILES):
            ids = eids[et == t]
            nv = len(ids)
            sl = slice(off, off + nv)
            SRC[sl] = src[ids]
            TLOCREL[sl] = (tloc[ids] - t * TN).astype(np.float32)
            TLOC[sl] = tloc[ids] - t * TN
            REC[sl] = recip_e[ids]
            SRH[0, sl] = cred_e[ids]
            oh = np.eye(4, dtype=np.float32)[etype[ids]]
            SRH[1:5, sl] = oh.T
            off += int(P[t])
        assert off == E_pad

        # images
        srcg_img = SRC.reshape(n_chunks, 128).T.astype(np.int32)  # [128, nch]
        tlr_img = TLOCREL.reshape(n_chunks, 128).T.copy()         # [128, nch]
        rec_img = REC.reshape(n_chunks, 128).T.copy()             # [128, nch]
        # apg idx image: per chunk [128, 8]; row p, col s = u[s*16 + p%16]
        apg = np.zeros((128, 8 * n_chunks), np.int16)
        for ch in range(n_chunks):
            u = TLOC[ch * 128:(ch + 1) * 128]
            blk = u.reshape(8, 16).T.astype(np.int16)
            apg[:, 8 * ch:8 * (ch + 1)] = np.tile(blk, (8, 1))
        # pass-1 src feed, pre-transposed per chunk: [nch, 128 D, 128 E]
        feed = h0[SRC].reshape(n_chunks, 128, D).transpose(0, 2, 1)
        feed = np.ascontiguousarray(feed).astype(BF16)

        hshT = np.zeros((D, cfg.NSHP), np.float32)
        hshT[:, :NSH] = h0[c * NSH:(c + 1) * NSH].T
        nomsg = np.zeros((1, cfg.NSHP), np.float32)
        nomsg[0, :NSH] = BIG * (cnt[c * NSH:(c + 1) * NSH] == 0)
        nomsg[0, NSH:] = BIG

        per_core.append(dict(
            srcg=srcg_img, tlr=tlr_img, recg=rec_img, apg=apg,
            srhs=np.ascontiguousarray(SRH.astype(BF16)), feed=feed.reshape(-1),
            hsh0T=hshT, nomsg=nomsg.astype(BF16)))

    W1 = np.asarray(inputs["W1"], np.float32)
    ee = np.asarray(inputs["edge_emb"], np.float32)
    bih = np.asarray(inputs["bih"], np.float32)
    bhh = np.asarray(inputs["bhh"], np.float32)
    shared = dict(
        W1srcT=np.ascontiguousarray(W1[:, :D].T).astype(BF16),
        W1tgtT=np.ascontiguousarray(W1[:, D:2 * D].T).astype(BF16),
        W1staT=np.ascontiguousarray(np.concatenate(
            [W1[:, 2 * D + 64][None, :],
             ee @ W1[:, 2 * D:2 * D + 64].T], 0)).astype(BF16),
        b1col=np.asarray(inputs["b1"], np.float32)[:, None],
        W2T=np.ascontiguousarray(
            np.asarray(inputs["W2"], np.float32).T).astype(BF16),
        b2col=np.asarray(inputs["b2"], np.float32)[:, None],
        WihT=np.ascontiguousarray(
            np.asarray(inputs["Wih"], np.float32).T).astype(BF16),
        WhhT=np.ascontiguousarray(
            np.asarray(inputs["Whh"], np.float32).T).astype(BF16),
        brzcol=np.ascontiguousarray((bih + bhh)[:2 * D].reshape(2, D).T),
        bnhcol=bhh[2 * D:][:, None].copy(),
        bnicol=bih[2 * D:][:, None].copy(),
        ones1=np.ones((1, D), BF16),
        eye=np.eye(D, dtype=np.float32),
        iota=np.tile(np.arange(TN, dtype=np.float32), (128, 1)),
    )
    meta = dict(P=P, E_pad=E_pad, n_chunks=n_chunks,
                tile_of_chunk=tile_of_chunk)
    return meta, per_core, shared


def _build(cfg, meta):
    global FAKE_SILU
    from concourse import bacc, tile, mybir
    import concourse.bass as bass

    nc = bacc.Bacc("TRN2", target_bir_lowering=False, debug=False,
                   num_devices=N_CORES)
    f32, i32, i16 = mybir.dt.float32, mybir.dt.int32, mybir.dt.int16
    bf16 = mybir.dt.bfloat16
    AF = mybir.ActivationFunctionType
    NSH, NSHP = cfg.NSH, cfg.NSHP
    P = meta["P"]
    n_chunks = meta["n_chunks"]
    E_pad = meta["E_pad"]

    srcg = nc.dram_tensor("srcg", [128, n_chunks], i32, kind="ExternalInput")
    tlr = nc.dram_tensor("tlr", [128, n_chunks], f32, kind="ExternalInput")
    recg = nc.dram_tensor("recg", [128, n_chunks], f32, kind="ExternalInput")
    apg = nc.dram_tensor("apg", [128, 8 * n_chunks], i16,
                         kind="ExternalInput")
    srhs = nc.dram_tensor("srhs", [5, E_pad], bf16, kind="ExternalInput")
    feed = nc.dram_tensor("feed", [E_pad * 128], bf16, kind="ExternalInput")
    hsh0T = nc.dram_tensor("hsh0T", [D, NSHP], f32, kind="ExternalInput")
    nomsg = nc.dram_tensor("nomsg", [1, NSHP], bf16,
                           kind="ExternalInput")
    wnames = dict(W1srcT=[D, D], W1tgtT=[D, D], W1staT=[5, D],
                  b1col=[D, 1], W2T=[D, D], b2col=[D, 1],
                  WihT=[D, 3 * D], WhhT=[D, 3 * D], brzcol=[D, 2],
                  bnhcol=[D, 1], bnicol=[D, 1], ones1=[1, D], eye=[D, D],
                  iota=[128, TN])
    wbf = {"W1srcT", "W1tgtT", "W1staT", "W2T", "WihT", "WhhT", "ones1"}
    wt = {k: nc.dram_tensor(k, sh, bf16 if k in wbf else f32,
                            kind="ExternalInput")
          for k, sh in wnames.items()}
    h_out = nc.dram_tensor("h_out", [NSHP, D], f32, kind="ExternalOutput")
    h1rm = nc.dram_tensor("h1rm", [NSHP, D], f32)
    h1full = nc.dram_tensor("h1full", [cfg.N, D], f32, addr_space="Shared")

    with tile.TileContext(nc) as tc:
        with (
            tc.tile_pool(name="const", bufs=1) as cpool,
            tc.tile_pool(name="sfeed", bufs=5) as fpool,
            tc.tile_pool(name="work", bufs=6) as wpool,
            tc.tile_pool(name="gru", bufs=1) as upool,
            tc.tile_pool(name="pt", bufs=2, space="PSUM") as pt,
            tc.tile_pool(name="pg", bufs=4, space="PSUM") as pg,
            tc.tile_pool(name="pu", bufs=2, space="PSUM") as pu,
        ):
            w = {}
            for k, sh in wnames.items():
                w[k] = cpool.tile(sh, bf16 if k in wbf else f32,
                                  tag=k, name=f"w_{k}")
                nc.sync.dma_start(out=w[k][:, :], in_=wt[k][:, :])
            slab = cpool.tile([D, NSHP], f32, tag="slab")
            for t in range(cfg.NTILES):
                nc.sync.dma_start(out=slab[:, TN * t:TN * (t + 1)],
                                  in_=hsh0T[:, TN * t:TN * (t + 1)])
            srcg_sb = cpool.tile([128, n_chunks], i32, tag="srcg")
            nc.sync.dma_start(out=srcg_sb[:, :], in_=srcg[:, :])
            tlr_sb = cpool.tile([128, n_chunks], f32, tag="tlr")
            nc.sync.dma_start(out=tlr_sb[:, :], in_=tlr[:, :])
            rec_sb = cpool.tile([128, n_chunks], f32, tag="rec")
            nc.sync.dma_start(out=rec_sb[:, :], in_=recg[:, :])
            apg_sb = cpool.tile([128, 8 * n_chunks], i16, tag="apg")
            nc.sync.dma_start(out=apg_sb[:, :], in_=apg[:, :])

            for p in range(N_PASSES):
                ch0 = 0
                for t in range(cfg.NTILES):
                    tch = int(P[t]) // 128
                    aggT = pg.tile([128, TN], f32, tag="pg", name=f"agg{p}_{t}")
                    for b0 in range(0, tch, 4):
                        bw = min(4, tch - b0)
                        cb = ch0 + b0
                        sT = wpool.tile([128, 512], bf16, tag="sT")
                        tT = wpool.tile([128, 512], bf16, tag="tT")
                        tTf = fpool.tile([128, 512], f32, tag="tTf")
                        if p == 0:
                            nc.sync.dma_start(
                                out=sT[:, :128 * bw]
                                .rearrange("p (a e) -> p a e", e=128),
                                in_=feed[cb * 128 * 128:
                                         (cb + bw) * 128 * 128]
                                .rearrange("(a p e) -> p a e", p=128, e=128))
                        for a in range(bw):
                            ch = cb + a
                            if p == 1:
                                gsr = fpool.tile([128, 128], f32, tag="gsr")
                                nc.gpsimd.indirect_dma_start(
                                    out=gsr[:, :], out_offset=None,
                                    in_=h1full[:, :],
                                    in_offset=bass.IndirectOffsetOnAxis(
                                        ap=srcg_sb[:, ch:ch + 1], axis=0))
                                ps = pt.tile([128, 512], f32, tag="pt",
                                             name=f"ps{p}_{ch}")
                                nc.tensor.transpose(
                                    ps[:, :128], gsr[:, :], w["eye"][:, :])
                                nc.vector.tensor_copy(
                                    sT[:, 128 * a:128 * (a + 1)], ps[:, :128])
                            nc.gpsimd.ap_gather(
                                tTf[:, 128 * a:128 * (a + 1)],
                                slab[:, TN * t:TN * (t + 1)],
                                apg_sb[:, 8 * ch:8 * (ch + 1)],
                                channels=128, num_elems=TN, d=1, num_idxs=128)
                        nc.vector.tensor_copy(tT[:, :128 * bw],
                                               tTf[:, :128 * bw])
                        y1 = pg.tile([128, 512], f32, tag="pg",
                                     name=f"y1_{p}_{cb}")
                        nc.tensor.matmul(y1[:, :128 * bw], w["W1srcT"][:, :],
                                         sT[:, :128 * bw],
                                         start=True, stop=False)
                        nc.tensor.matmul(y1[:, :128 * bw], w["W1tgtT"][:, :],
                                         tT[:, :128 * bw],
                                         start=False, stop=False)
                        srh_t = fpool.tile([5, 512], bf16, tag="srh")
                        nc.sync.dma_start(
                            out=srh_t[:, :128 * bw],
                            in_=srhs[:, 128 * cb:128 * (cb + bw)])
                        nc.tensor.matmul(
                            y1[:, :128 * bw], w["W1staT"][:, :],
                            srh_t[:, :128 * bw],
                            start=False, stop=True)
                        y1s = wpool.tile([128, 512], bf16, tag="y1s")
                        if FAKE_SILU:
                            zb = wpool.tile([128, 512], f32, tag="zb")
                            nc.scalar.activation(zb[:, :128 * bw],
                                                 y1[:, :128 * bw],
                                                 AF.Identity,
                                                 bias=w["b1col"][:, 0:1])
                            sg = wpool.tile([128, 512], f32, tag="sg")
                            nc.scalar.activation(sg[:, :128 * bw],
                                                 y1[:, :128 * bw],
                                                 AF.Sigmoid,
                                                 bias=w["b1col"][:, 0:1])
                            nc.vector.tensor_mul(y1s[:, :128 * bw],
                                                 zb[:, :128 * bw],
                                                 sg[:, :128 * bw])
                        else:
                            nc.scalar.activation(y1s[:, :128 * bw],
                                                 y1[:, :128 * bw], AF.Silu,
                                                 bias=w["b1col"][:, 0:1])
                        for a in range(bw):
                            ch = cb + a
                            y2 = pt.tile([128, 512], f32, tag="pt",
                                         name=f"y2_{p}_{ch}")
                            nc.tensor.matmul(
                                y2[:, :128], y1s[:, 128 * a:128 * (a + 1)],
                                w["W2T"][:, :], start=True, stop=True)
                            mp = wpool.tile([128, 128], bf16, tag="mp")
                            nc.vector.tensor_copy(mp[:, :], y2[:, :128])
                            S = wpool.tile([128, TN], bf16, tag="S")
                            nc.vector.tensor_scalar(
                                out=S[:, :], in0=w["iota"][:, :],
                                scalar1=tlr_sb[:, ch:ch + 1],
                                scalar2=rec_sb[:, ch:ch + 1],
                                op0=mybir.AluOpType.is_equal,
                                op1=mybir.AluOpType.mult)
                            nc.tensor.matmul(aggT[:, :], mp[:, :], S[:, :],
                                             start=(b0 == 0 and a == 0),
                                             stop=(b0 + 4 >= tch
                                                   and a == bw - 1))
                    ch0 += tch
                    # ---- GRU for this node tile (in T layout)
                    cl, chh = TN * t, TN * (t + 1)
                    xT = upool.tile([128, TN], bf16, tag="xT")
                    nc.scalar.activation(xT[:, :], aggT[:, :], AF.Identity,
                                         bias=w["b2col"][:, 0:1])
                    hTs = slab[:, cl:chh]
                    hTb = upool.tile([128, TN], bf16, tag="hTb")
                    nc.vector.tensor_copy(hTb[:, :], hTs)
                    pr = pu.tile([128, TN], f32, tag="pu", name=f"pr{p}_{t}")
                    pz = pu.tile([128, TN], f32, tag="pu", name=f"pz{p}_{t}")
                    nc.tensor.matmul(pr[:, :], w["WihT"][:, 0:D], xT[:, :],
                                     start=True, stop=False)
                    nc.tensor.matmul(pr[:, :], w["WhhT"][:, 0:D], hTb[:, :],
                                     start=False, stop=True)
                    nc.tensor.matmul(pz[:, :], w["WihT"][:, D:2 * D],
                                     xT[:, :], start=True, stop=False)
                    nc.tensor.matmul(pz[:, :], w["WhhT"][:, D:2 * D], hTb[:, :],
                                     start=False, stop=False)
                    nm_t = fpool.tile([1, TN], bf16, tag="nm")
                    nc.sync.dma_start(out=nm_t[:, :], in_=nomsg[:, cl:chh])
                    nc.tensor.matmul(pz[:, :], w["ones1"][:, :],
                                     nm_t[:, :], start=False, stop=True)
                    r_s = upool.tile([128, TN], f32, tag="r_s")
                    nc.scalar.activation(r_s[:, :], pr[:, :], AF.Sigmoid,
                                         bias=w["brzcol"][:, 0:1])
                    z_s = upool.tile([128, TN], f32, tag="z_s")
                    nc.scalar.activation(z_s[:, :], pz[:, :], AF.Sigmoid,
                                         bias=w["brzcol"][:, 1:2])
                    pni = pu.tile([128, TN], f32, tag="pu", name=f"pi{p}_{t}")
                    pnh = pu.tile([128, TN], f32, tag="pu", name=f"ph{p}_{t}")
                    nc.tensor.matmul(pni[:, :], w["WihT"][:, 2 * D:3 * D],
                                     xT[:, :], start=True, stop=True)
                    nc.tensor.matmul(pnh[:, :], w["WhhT"][:, 2 * D:3 * D],
                                     hTb[:, :], start=True, stop=True)
                    ghn = upool.tile([128, TN], f32, tag="ghn")
                    nc.scalar.activation(ghn[:, :], pnh[:, :], AF.Identity,
                                         bias=w["bnhcol"][:, 0:1])
                    t1 = upool.tile([128, TN], f32, tag="t1")
                    nc.vector.tensor_mul(t1[:, :], r_s[:, :], ghn[:, :])
                    t2 = upool.tile([128, TN], f32, tag="t2")
                    nc.vector.tensor_add(t2[:, :], pni[:, :], t1[:, :])
                    n_s = upool.tile([128, TN], f32, tag="n_s")
                    nc.scalar.activation(n_s[:, :], t2[:, :], AF.Tanh,
                                         bias=w["bnicol"][:, 0:1])
                    d_s = upool.tile([128, TN], f32, tag="d_s")
                    nc.vector.tensor_sub(d_s[:, :], hTs, n_s[:, :])
                    zd = upool.tile([128, TN], f32, tag="zd")
                    nc.vector.tensor_mul(zd[:, :], z_s[:, :], d_s[:, :])
                    hn = upool.tile([128, TN], f32, tag="hn")
                    nc.vector.tensor_add(hn[:, :], n_s[:, :], zd[:, :])
                    # write back into the resident slab (h for next pass)
                    nc.vector.tensor_copy(slab[:, cl:chh], hn[:, :])
                    # back-transpose to row-major for AllGather / output
                    hrows = upool.tile([128, TN], f32, tag="hrows")
                    pb = pt.tile([128, 512], f32, tag="pt", name=f"pb{p}_{t}")
                    for a in range(4):
                        nc.tensor.transpose(
                            pb[:, 128 * a:128 * (a + 1)],
                            hn[:, 128 * a:128 * (a + 1)], w["eye"][:, :])
                        nc.vector.tensor_copy(
                            hrows[:, 128 * a:128 * (a + 1)],
                            pb[:, 128 * a:128 * (a + 1)])
                    dst = h1rm if p == 0 else h_out
                    nc.sync.dma_start(
                        out=dst[cl:chh, :].rearrange("(a q) d -> q a d",
                                                     q=128),
                        in_=hrows[:, :].rearrange("q (a d) -> q a d", d=128))
                if p == 0:
                    nc.gpsimd.collective_compute(
                        "AllGather", mybir.AluOpType.bypass,
                        replica_groups=[list(range(N_CORES))],
                        ins=[h1rm[0:NSH, :]],
                        outs=[h1full[:, :]])
    nc.compile()
    return nc


def build_and_run(inputs, cfg=None, sim=False, trace=False, tmpdir=None):
    global FAKE_SILU
    cfg = cfg or _Cfg()
    meta, per_core, shared = _plan(cfg, inputs)
    FAKE_SILU = bool(sim)
    nc = _build(cfg, meta)
    maps = []
    for c in range(N_CORES):
        m = {k: np.ascontiguousarray(v) for k, v in per_core[c].items()}
        m.update({k: np.ascontiguousarray(v) for k, v in shared.items()})
        maps.append(m)
    if sim:
        from concourse.bass_interp import MultiCoreSim
        ms = MultiCoreSim(nc, num_cores=N_CORES, trace=False)
        for c in range(N_CORES):
            for k, v in maps[c].items():
                ms.cores[c].tensor(k)[:] = v
        ms.simulate(check_with_hw=False)
        shards = [np.array(ms.cores[c].tensor("h_out"))[:cfg.NSH]
                  for c in range(N_CORES)]
        return np.concatenate(shards, axis=0), None
    from concourse import bass_utils
    res = bass_utils.run_bass_kernel_spmd(
        nc, maps, list(range(N_CORES)), trace=trace, tmpdir=tmpdir)
    shards = [res.results[c]["h_out"][:cfg.NSH] for c in range(N_CORES)]
    return np.concatenate(shards, axis=0), res


def kernel(**inputs):
    out, _ = build_and_run(inputs)
    return out.astype(np.float32)
